# revision 21
# baseline (speedup 1.0000x reference)
"""Trainium2 Bass kernel for the SE-gated Non-local block (rank-1 attention).

Math (per batch item b, x viewed as [C, N] with N = H*W):
    S[c]    = sum_n x[c, n]                      (spatial sum)
    hid     = relu((se_w1 / N) @ S + se_b1)      (SE bottleneck; 1/N folds the mean)
    gate    = sigmoid(w2e @ [hid; 1])            (se_b2 folded in as an extra w2 row)
    w5e     = gate * [theta_w | 0 | 0 | g_w | phi_w]  [C, 5]
    prow    = w5e.T @ x + [th_b, 1, 1, g_b, phi_b]    [5, N]
              rows: theta, ONES, ONES, g, phi   (the ones rows come from the bias)
    s_raw   = sum_n prow[3] * prow[4]
    out     = x + As (outer) theta + (Bc_hi + Bc_lo) (outer) ones   where
              inv = bn_gamma / sqrt(bn_var + eps)
              As  = (W_w * inv / N) * s_raw      (1/N folds the f/N normalizer)
              Bc  = (W_b - bn_mean) * inv + bn_beta,  split hi/lo bf16.

Schedule (v3): HBM traffic is the floor (37.7 MB/core at ~410 GB/s observed
= ~92 us of DMA time).  Loads AND stores ping-pong the two HWDGE rings
(sync/scalar) so each transfer's ~2.3 us completion-receipt bubble overlaps
the other ring's data (single-ring sequential DMAs serialize those bubbles -
measured +8 us on loads alone).

- x is resident in SBUF as bf16; f32 loads land in a transient 4-buf pool and
  are released by the fused cast+rowsum pass.  Output = bf16(x) + corr:
  ~1.6e-3 rel err vs the 2e-2 gate, and both items' bf16 tiles coexist so
  item1's casts never wait on item0's projections.
- Cast+rowsum: item0's even chunks go on DVE (tensor_scalar with accum_out),
  odd chunks on ACT - the pair of chunks that lands together is processed in
  parallel, so item0's gate is ready ~4 us earlier.  Item1's casts all on ACT
  (DVE is busy with prow copies by then).
- SE: php accumulates on PE per chunk as rowsums land; b2 is folded into the
  gate matmul as an extra stationary row against a constant-1 row in hid1.
- proj: 36 bf16 matmuls [128x5x512]; psum->prow copies on DVE (bias fused).
  g.phi dot: SWDGE reshape [1,1536]->[128,12] groups as produced (DVE lanes
  cannot cross partitions), then mul+rowsum, cross-partition total via
  ones-matmul, ab3 row 0 = art * s_raw.
- corr = [As; Bc_hi; Bc_lo].T @ [theta; 1; 1] on PE into [128,1024] psum
  tiles (2 mms each, bank-limited); DVE adds out = bf16(x) + psum per 1024.
- PE emission order matches runtime readiness (in-order engines!):
  php0, gate0, proj0, sb0, corr0[k0,k1], php1, gate1, corr0[k2,k3], proj1,
  sb1, corr1 - item1's SE+proj interleave item0's correction, and stores
  flow per chunk behind the adds on the ping-ponged rings.
"""

import contextlib

import numpy as np

B, C, H, W = 16, 512, 96, 48
N = H * W            # 4608
P = 128
KC = C // P          # 4 channel chunks
NB = 512             # proj free-dim block = one fp32 PSUM bank
NJ = N // NB         # 9
CB = 1024            # corr psum tile width (2 banks; 2 matmuls + 1 add)
NCORES = 8
BPC = B // NCORES    # 2 batch items per core
SE_C = C // 16       # 32
MR = N // P          # 36: elems per partition in the reshaped g/phi rows
BN_EPS = 1e-5

_CACHE = {}
LAST_RESULTS = None


def _build_bass():
    import concourse.mybir as mybir
    from concourse.bacc import Bacc
    from concourse.tile import TileContext

    f32 = mybir.dt.float32
    bf16 = mybir.dt.bfloat16
    AF = mybir.ActivationFunctionType
    ALU = mybir.AluOpType
    AX = mybir.AxisListType

    nc = Bacc()
    xs = nc.dram_tensor("xs", [BPC, C, N], f32, kind="ExternalInput")
    w1 = nc.dram_tensor("w1", [P, KC, SE_C], f32, kind="ExternalInput")
    w2e = nc.dram_tensor("w2e", [SE_C + 1, C], f32, kind="ExternalInput")
    b1 = nc.dram_tensor("b1", [SE_C, 1], f32, kind="ExternalInput")
    w5 = nc.dram_tensor("w5", [P, KC, 5], f32, kind="ExternalInput")
    pb = nc.dram_tensor("pb", [5, 1], f32, kind="ExternalInput")
    ar = nc.dram_tensor("ar", [1, C], f32, kind="ExternalInput")    # W_w*inv/N
    bchl = nc.dram_tensor("bchl", [2, C], bf16, kind="ExternalInput")  # Bc hi/lo
    out_d = nc.dram_tensor("out", [BPC, C, N], f32, kind="ExternalOutput")

    with TileContext(nc) as tc:
        with (
            tc.tile_pool(name="wpool", bufs=1) as wpool,
            tc.tile_pool(name="ldpool", bufs=6) as ldpool,
            tc.tile_pool(name="xbpool", bufs=BPC * KC) as xbpool,
            tc.tile_pool(name="opool", bufs=4) as opool,
            tc.tile_pool(name="ppool", bufs=1) as ppool,
            tc.tile_pool(name="spool", bufs=2) as spool,
            tc.tile_pool(name="ps_se", bufs=2, space="PSUM") as ps_se,
            tc.tile_pool(name="ps_pj", bufs=2, space="PSUM") as ps_pj,
            tc.tile_pool(name="ps_cr", bufs=2, space="PSUM") as ps_cr,
        ):
            w1t = wpool.tile([P, KC, SE_C], f32, tag="w1t")
            w2t = wpool.tile([SE_C + 1, C], f32, tag="w2t")
            b1t = wpool.tile([SE_C, 1], f32, tag="b1t")
            w5t = wpool.tile([P, KC, 5], f32, tag="w5t")
            pbt = wpool.tile([5, 1], f32, tag="pbt")
            art = wpool.tile([1, C], f32, tag="art")
            ab3 = wpool.tile([3, C], bf16, tag="ab3")     # rows: As, Bc_hi, Bc_lo
            hid1 = wpool.tile([SE_C + 1, 1], f32, tag="hid1")  # [hid; 1.0]
            on128 = wpool.tile([P, P], f32, tag="on128")  # all-ones (part. sum)

            nc.vector.memset(hid1[SE_C:SE_C + 1, :], 1.0)
            nc.vector.memset(on128[:], 1.0)
            for t, d in ((w1t, w1), (w2t, w2e), (b1t, b1),
                         (w5t, w5), (pbt, pb), (art, ar)):
                nc.gpsimd.dma_start(out=t[:], in_=d[:])
            nc.gpsimd.dma_start(out=ab3[1:3, :], in_=bchl[:])

            # preload the ACT sigmoid table while idle (else the first SE
            # sigmoid pays a ~1.3us ACT_TABLE_LOAD on the critical path)
            dmy = spool.tile([1, 1], f32, tag="dmy", bufs=1)
            nc.vector.memset(dmy[:], 0.0)
            nc.scalar.activation(out=dmy[:], in_=dmy[:], func=AF.Sigmoid)

            # ---- all 8 chunk loads, ping-ponging the two HWDGE rings ----
            xts = []
            for i in range(BPC * KC):
                b, k = divmod(i, KC)
                xt = ldpool.tile([P, N], f32, tag="xt")
                eng = nc.sync if i % 2 == 0 else nc.scalar
                eng.dma_start(out=xt[:], in_=xs[b, k * P:(k + 1) * P, :])
                xts.append(xt)

            xbs = [[None] * KC for _ in range(BPC)]
            xps = [None] * BPC
            prows = [None] * BPC

            def cast_one(b, k, eng_dve):
                # fused bf16 cast + rowsum of chunk k into xp[:, k]
                xb = xbpool.tile([P, N], bf16, tag="xb", name="xb")
                if eng_dve:
                    nc.vector.tensor_scalar(
                        out=xb[:], in0=xts[b * KC + k][:], scalar1=0.0,
                        scalar2=0.0, op0=ALU.add, op1=ALU.add,
                        accum_out=xps[b][:, k:k + 1])
                else:
                    nc.scalar.activation(out=xb[:], in_=xts[b * KC + k][:],
                                         func=AF.Identity,
                                         accum_out=xps[b][:, k:k + 1])
                xbs[b][k] = xb

            def se_php(b, ks):
                for k in ks:
                    nc.tensor.matmul(phps[b][:], w1t[:, k, :],
                                     xps[b][:, k:k + 1],
                                     start=(k == 0), stop=(k == KC - 1))

            def se_gate(b):
                nc.scalar.activation(out=hid1[0:SE_C, :], in_=phps[b][:],
                                     func=AF.Relu, bias=b1t[:], scale=1.0)
                gate = spool.tile([P, KC], f32, tag="gate", name="gate")
                for k in range(KC):
                    gp = ps_se.tile([P, 1], f32, tag="ps_se", name="gp")
                    nc.tensor.matmul(gp[:], w2t[:, k * P:(k + 1) * P],
                                     hid1[:], start=True, stop=True)
                    nc.scalar.activation(out=gate[:, k:k + 1], in_=gp[:],
                                         func=AF.Sigmoid)
                w5e = spool.tile([P, KC, 5], bf16, tag="w5e", name="w5e")
                for k in range(KC):
                    nc.vector.tensor_scalar_mul(out=w5e[:, k, :],
                                                in0=w5t[:, k, :],
                                                scalar1=gate[:, k:k + 1])
                return w5e

            def proj_and_dot(b, w5e, jbase=None, jstep=1.3):
                # prow = w5e.T @ x (bf16 PE); psum->prow copies on DVE with
                # the bias fused; g/phi rows stream into [128, .] layout as
                # they are produced (SWDGE; same n-permutation for both rows)
                prow = ppool.tile([5, N], bf16, tag="prow", name="prow")
                prows[b] = prow
                g_rs = spool.tile([P, MR], bf16, tag="g_rs", name="g_rs")
                p_rs = spool.tile([P, MR], bf16, tag="p_rs", name="p_rs")
                for j in range(NJ):
                    with (tc.tile_wait_until((jbase + jstep * j) / 1000.0)
                          if jbase is not None else contextlib.nullcontext()):
                        pp = ps_pj.tile([5, NB], f32, tag="pp", name="pp")
                        for k in range(KC):
                            nc.tensor.matmul(
                                pp[:], w5e[:, k, :],
                                xbs[b][k][:, j * NB:(j + 1) * NB],
                                start=(k == 0), stop=(k == KC - 1))
                        nc.vector.tensor_scalar_add(
                            out=prow[:, j * NB:(j + 1) * NB],
                            in0=pp[:], scalar1=pbt[:])
                        if j in (3, 7, 8):
                            lo = {3: 0, 7: 4, 8: 8}[j]
                            nsl = slice(lo * NB, (j + 1) * NB)
                            msl = slice(lo * (NB // P), (j + 1) * (NB // P))
                            nc.gpsimd.dma_start(out=g_rs[:, msl],
                                                in_=prow[3:4, nsl])
                            nc.gpsimd.dma_start(out=p_rs[:, msl],
                                                in_=prow[4:5, nsl])
                # dot: prod = g*phi, r1 = rowsum(prod), cross-partition sum
                prod = spool.tile([P, MR], f32, tag="prod", name="prod")
                r1 = spool.tile([P, 1], f32, tag="r1", name="r1")
                nc.vector.tensor_mul(out=prod[:], in0=g_rs[:], in1=p_rs[:])
                nc.vector.reduce_sum(out=r1[:], in_=prod[:], axis=AX.X)
                rs1[b] = r1

            def sb_scale(b):
                sb = ps_se.tile([P, 1], f32, tag="ps_se", name="sb")
                nc.tensor.matmul(sb[:], on128[:], rs1[b][:], start=True,
                                 stop=True)
                # As row = (W_w*inv/N) * s_raw, into ab3 row 0 (bf16)
                nc.vector.tensor_scalar_mul(out=ab3[0:1, :], in0=art[:],
                                            scalar1=sb[0:1, 0:1])

            def corr_chunk(b, k):
                # corr = As x theta + Bc x ones via PE (3-row bf16), then
                # out = bf16(x) + corr IN PLACE into the resident xb tile.
                # Two add paths, balanced so no single engine serializes:
                #  - blocks 0,1: DVE adds straight from psum (1x, ~1.15us)
                #  - blocks 2,3,4: ACT copies psum->bf16 SBUF (ACT is
                #    otherwise idle here), then DVE 2x all-SBUF bf16 add
                # The store is a SWDGE cast-DMA bf16 SBUF -> f32 HBM; no
                # f32 out tiles at all.
                prow = prows[b]
                for gi, n0 in enumerate(range(0, N, CB)):
                    gw = min(CB, N - n0)
                    cp = ps_cr.tile([P, CB], f32, tag="cp", name="cp")
                    for m0 in range(0, gw, NB):
                        nc.tensor.matmul(cp[:, m0:m0 + NB],
                                         ab3[:, k * P:(k + 1) * P],
                                         prow[0:3, n0 + m0:n0 + m0 + NB],
                                         start=True, stop=True)
                    if gi < 2:
                        nc.vector.tensor_add(out=xbs[b][k][:, n0:n0 + gw],
                                             in0=xbs[b][k][:, n0:n0 + gw],
                                             in1=cp[:, 0:gw])
                    else:
                        crs = opool.tile([P, CB], bf16, tag="crs", name="crs")
                        nc.scalar.activation(out=crs[:, 0:gw],
                                             in_=cp[:, 0:gw],
                                             func=AF.Identity)
                        nc.vector.tensor_add(out=xbs[b][k][:, n0:n0 + gw],
                                             in0=xbs[b][k][:, n0:n0 + gw],
                                             in1=crs[:, 0:gw])
                nc.gpsimd.dma_start(out=out_d[b, k * P:(k + 1) * P, :],
                                    in_=xbs[b][k][:])

            # shared small state
            xps[0] = spool.tile([P, KC], f32, tag="xp", name="xp0")
            xps[1] = spool.tile([P, KC], f32, tag="xp", name="xp1")
            phps = [ps_se.tile([SE_C, 1], f32, tag="ps_se", name=f"php{b}")
                    for b in range(BPC)]
            rs1 = [None, None]

            # ---- manual schedule floors (pseudo-us in the scheduler's
            # pass-1 sim).  The Tile scheduler bakes each engine's stream
            # order from a cost-model simulation whose DMA timing is far
            # more optimistic than hardware; without floors it bakes
            # item1's 4us casts ahead of item0's SE sigmoids on ACT and the
            # in-order engine then stalls ~7us waiting for item1's chunks.
            # Floors are LOWER bounds in sim time only - hardware still
            # runs purely dependency-paced, these just pin the per-engine
            # instruction order to match HW readiness order. ----
            def at(t_us):
                return tc.tile_wait_until(t_us / 1000.0)

            # item 0 head: casts split DVE/ACT so the landing pair is
            # processed in parallel; php accumulates during loads
            land0 = (12, 12, 24, 24)   # chunk-pair land times (HW-measured)
            for k in range(KC):
                with at(land0[k]):
                    cast_one(0, k, eng_dve=(k % 2 == 0))
                    se_php(0, [k])
            with at(36):
                w5e0 = se_gate(0)
            with at(38):
                proj_and_dot(0, w5e0)

            # item 1 casts: mostly ACT; c12 on DVE (idle slot between dot0
            # and the corr adds) so the last landing pair casts in parallel
            land1 = (45, 47, 56, 57)
            for k in range(KC):
                with at(land1[k]):
                    cast_one(1, k, eng_dve=(k == 2))

            with at(53):
                sb_scale(0)
            with at(54):
                corr_chunk(0, 0)
            with at(58.5):
                corr_chunk(0, 1)
            # item1 SE on PE between item0's correction chunks
            with at(62.5):
                se_php(1, range(KC))
            with at(63):
                w5e1 = se_gate(1)
            with at(63.5):
                corr_chunk(0, 2)
            # proj1 j-blocks interleave corr0's last chunk so neither PE
            # nor DVE starves (copies1 must not queue behind all of adds0)
            proj_and_dot(1, w5e1, jbase=68, jstep=1.2)
            with at(73):
                corr_chunk(0, 3)
            with at(81):
                sb_scale(1)
            for k in range(KC):
                with at(82 + 4.5 * k):
                    corr_chunk(1, k)

    nc.finalize()
    return nc


def kernel(**inputs):
    global LAST_RESULTS
    from concourse.bass_utils import run_bass_kernel_spmd
    import ml_dtypes

    a = {k: np.asarray(v, dtype=np.float32) for k, v in inputs.items()}
    x = np.ascontiguousarray(a["x"]).reshape(B, C, N)

    inv = a["bn_gamma"] / np.sqrt(a["bn_var"] + BN_EPS)
    A = (a["W_w"] * inv / N).astype(np.float32)
    Bc = ((a["W_b"] - a["bn_mean"]) * inv + a["bn_beta"]).astype(np.float32)
    Bc_hi = Bc.astype(ml_dtypes.bfloat16)
    Bc_lo = (Bc - Bc_hi.astype(np.float32)).astype(ml_dtypes.bfloat16)

    w1h = np.ascontiguousarray(
        (a["se_w1"] / N).T.reshape(KC, P, SE_C).transpose(1, 0, 2)).astype(np.float32)
    w2h = np.ascontiguousarray(
        np.vstack([a["se_w2"].T, a["se_b2"][None, :]])).astype(np.float32)
    b1h = np.ascontiguousarray(a["se_b1"].reshape(SE_C, 1))
    zc = np.zeros(C, np.float32)
    w5h = np.ascontiguousarray(
        np.stack([a["theta_w"], zc, zc, a["g_w"], a["phi_w"]],
                 axis=1).reshape(KC, P, 5).transpose(1, 0, 2)).astype(np.float32)
    pbh = np.array([[a["theta_b"]], [1.0], [1.0], [a["g_b"]], [a["phi_b"]]],
                   dtype=np.float32)
    arh = np.ascontiguousarray(A.reshape(1, C))
    bchlh = np.ascontiguousarray(np.stack([Bc_hi, Bc_lo], axis=0))

    if "nc" not in _CACHE:
        _CACHE["nc"] = _build_bass()
    nc = _CACHE["nc"]

    in_maps = []
    for c in range(NCORES):
        in_maps.append({
            "xs": np.ascontiguousarray(x[c * BPC:(c + 1) * BPC]),
            "w1": w1h, "w2e": w2h, "b1": b1h,
            "w5": w5h, "pb": pbh, "ar": arh, "bchl": bchlh,
        })

    res = run_bass_kernel_spmd(nc, in_maps, core_ids=list(range(NCORES)))
    LAST_RESULTS = res

    out = np.concatenate([res.results[c]["out"] for c in range(NCORES)], axis=0)
    return np.ascontiguousarray(out.reshape(B, C, H, W))
